# revision 1
# baseline (speedup 1.0000x reference)
"""Trainium2 Bass kernel: batched inverse of homogeneous affine transforms.

Problem: trf (B, 3, 4) fp32 "shift" affines. Padded M = [[I3 + dA, t], [0, 1]].
Output = top 3 rows of M^-1 = [A^-1 | -A^-1 t] where A = I3 + dA.

Closed form via the column-cross-product adjugate:
    inv(A) row r = (1/det) * cross(a_{r+1}, a_{r+2})   (columns a1,a2,a3, cyclic)
    det          = a1 . cross(a2, a3)
    col3_r       = -sum_j inv(A)[r, j] * t_j

Everything is elementwise over the batch -> memory-bound. The batch is
sharded over 8 NeuronCores; each core streams (BL, 12) fp32 in and out.

Per-core layout: chunks of 128 partitions x C matrices; the SBUF input tile
is (128, 12*C) with each partition holding C consecutive 12-float matrices.
All compute uses strided/broadcast access patterns directly on the
interleaved layout (fp32 tensor ops on DVE run at 1x regardless of stride).
Work is split across DVE (products/scale), GPSIMD (contiguous adds/subs)
and ACT (diag +1, reciprocal).
"""

import numpy as np

B = 4_194_304
NCORES = 8
BL = B // NCORES  # 524288 matrices per core
P = 128
C = 512  # matrices per partition per chunk


def _V(base_ap, off, dims):
    """Build a strided view of a tile: dims = [(step, count), ...] free dims,
    iterated with the LAST dim innermost. Offset in elements."""
    import concourse.bass as bass

    return bass.AP(
        base_ap.tensor,
        base_ap.offset + off,
        [list(base_ap.ap[0])] + [[int(s), int(n)] for s, n in dims],
    )


# default engine plan: op -> "v" (DVE) / "g" (GPSIMD)
DEFAULT_PLAN = {
    **{f"prod{k}": "v" for k in range(18)},
    "zsub": "g",
    "tm": "g",
    "det1": "g",
    "det2": "g",
    "scale9": "v",
    "w": "v",
    "s1": "g",
    "s2": "g",
}

# Products: (left position, right position) in the 12-float group.
# Positions: a=0 b=1 c=2 t0=3 d=4 e=5 f=6 t1=7 g=8 h=9 i=10 t2=11
# P[3r+j] = x_r[(j+1)%3] * y_r[(j+2)%3], Q[3r+j] = x_r[(j+2)%3] * y_r[(j+1)%3]
# with (x_r, y_r) = (a2,a3), (a3,a1), (a1,a2); cols a1=(0,4,8) a2=(1,5,9) a3=(2,6,10)
PRODS = [
    (5, 10), (9, 2), (1, 6),    # P, r=0: cross(a2,a3)
    (6, 8), (10, 0), (2, 4),    # P, r=1: cross(a3,a1)
    (4, 9), (8, 1), (0, 5),     # P, r=2: cross(a1,a2)
    (9, 6), (1, 10), (5, 2),    # Q, r=0
    (10, 4), (2, 8), (6, 0),    # Q, r=1
    (8, 5), (0, 9), (4, 1),     # Q, r=2
]


def build_nc(bl=BL, c=C, plan=None):
    import concourse.bass as bass
    import concourse.bacc as bacc
    import concourse.mybir as mybir
    from concourse.tile import TileContext

    plan = dict(DEFAULT_PLAN, **(plan or {}))
    f32 = mybir.dt.float32
    nch = bl // (P * c)
    assert bl == nch * P * c

    # Bacc (not plain Bass): Tile emits multi-wait instructions; Bacc's
    # generate_event_semaphores splits them to satisfy TRN2's 1-wait limit.
    nc = bacc.Bacc()
    trf = nc.declare_dram_parameter("trf", [bl, 12], f32, isOutput=False)
    out = nc.declare_dram_parameter("out", [bl, 12], f32, isOutput=True)
    trf_t = trf.ap().rearrange("(n p c) m -> n p (c m)", p=P, c=c)
    out_t = out.ap().rearrange("(n p c) m -> n p (c m)", p=P, c=c)

    with TileContext(nc) as tc:
        with (
            tc.tile_pool(name="io", bufs=2) as io,
            tc.tile_pool(name="tmp", bufs=1) as tmp,
        ):
            for n in range(nch):
                eng = {"v": nc.vector, "g": nc.gpsimd}

                tin = io.tile([P, 12 * c], f32, tag="tin")
                nc.sync.dma_start(out=tin[:], in_=trf_t[n])

                # diag += 1 in-place: positions {0,5,10} = stride 5
                dg = _V(tin, 0, [(12, c), (5, 3)])
                nc.scalar.add(dg, dg, 1.0)

                # P/Q products: pq planes 0-8 = P (cross components Z before
                # subtraction), planes 9-17 = Q; plane k = C contiguous floats
                pq = tmp.tile([P, 18 * c], f32, tag="pq")
                for k, (l, r) in enumerate(PRODS):
                    e = eng[plan[f"prod{k}"]]
                    e.tensor_mul(
                        _V(pq, k * c, [(1, c)]),
                        _V(tin, l, [(12, c)]),
                        _V(tin, r, [(12, c)]),
                    )

                # Z = P - Q (in place over P), flat 9C, contiguous
                pf = _V(pq, 0, [(1, 9 * c)])
                qf = _V(pq, 9 * c, [(1, 9 * c)])
                eng[plan["zsub"]].tensor_sub(pf, pf, qf)

                # det = a1 . Z[0:3]:  tm = a1 * Z3 ; det = tm0+tm1+tm2
                tm = tmp.tile([P, 3 * c], f32, tag="tm")
                # iteration (k, c): in0 strided tin cols, in1 Z planes, out tm
                eng[plan["tm"]].tensor_mul(
                    _V(tm, 0, [(c, 3), (1, c)]),
                    _V(tin, 0, [(4, 3), (12, c)]),
                    _V(pq, 0, [(c, 3), (1, c)]),
                )
                det = tmp.tile([P, c], f32, tag="det")
                eng[plan["det1"]].tensor_add(
                    det[:], _V(tm, 0, [(1, c)]), _V(tm, c, [(1, c)])
                )
                eng[plan["det2"]].tensor_add(det[:], det[:], _V(tm, 2 * c, [(1, c)]))

                # rdet = 1/det: ~2 ULP, two custom-DVE ops (det ~ 1, no edge
                # cases). Replicated to 3 planes (ISA ops are <=3D and don't
                # take 0-step broadcast APs).
                rdet3 = tmp.tile([P, 3 * c], f32, tag="rdet3")
                rscr = tmp.tile([P, c], f32, tag="rscr")
                nc.vector.reciprocal_approx_accurate(
                    _V(rdet3, 0, [(1, c)]), det[:], rscr[:]
                )
                nc.scalar.copy(_V(rdet3, c, [(1, c)]), _V(rdet3, 0, [(1, c)]))
                nc.scalar.copy(_V(rdet3, 2 * c, [(1, c)]), _V(rdet3, 0, [(1, c)]))

                # out 3x3 block: tout[4r+j] = Z[3r+j] * rdet  (one op per row,
                # iteration (c, j), all operands 3D)
                tout = io.tile([P, 12 * c], f32, tag="tout")
                for r in range(3):
                    eng[plan["scale9"]].tensor_mul(
                        _V(tout, 4 * r, [(12, c), (1, 3)]),
                        _V(pq, 3 * r * c, [(1, c), (c, 3)]),
                        _V(rdet3, 0, [(1, c), (c, 3)]),
                    )

                # W[r,j] = (tout[4r+j] * -1) * t_j  (scalar_tensor_tensor,
                # one per row); W lives in the dead Q region
                for r in range(3):
                    eng[plan["w"]].scalar_tensor_tensor(
                        _V(pq, (9 + 3 * r) * c, [(1, c), (c, 3)]),
                        _V(tout, 4 * r, [(12, c), (1, 3)]),
                        -1.0,
                        _V(tin, 3, [(12, c), (4, 3)]),
                        mybir.AluOpType.mult,
                        mybir.AluOpType.mult,
                    )

                # col3_r = W[r,0] + W[r,1] + W[r,2] -> tout positions {3,7,11}
                s = tmp.tile([P, 3 * c], f32, tag="s")
                eng[plan["s1"]].tensor_add(
                    _V(s, 0, [(c, 3), (1, c)]),
                    _V(pq, 9 * c, [(3 * c, 3), (1, c)]),
                    _V(pq, 10 * c, [(3 * c, 3), (1, c)]),
                )
                eng[plan["s2"]].tensor_add(
                    _V(tout, 3, [(4, 3), (12, c)]),
                    _V(s, 0, [(c, 3), (1, c)]),
                    _V(pq, 11 * c, [(3 * c, 3), (1, c)]),
                )

                nc.sync.dma_start(out=out_t[n], in_=tout[:])

    return nc


_CACHE = {}


def _get_nc():
    if "nc" not in _CACHE:
        nc = build_nc()
        # Bacc.finalize runs the bacc pipeline (event-sem wait splitting,
        # register allocation, ...); the PJRT path executes it as-is.
        nc.finalize()
        _CACHE["nc"] = nc
    return _CACHE["nc"]


def run(trf, trace=False, **spmd_kwargs):
    """Shard, run on 8 cores, gather. Returns (output, BassKernelResults)."""
    from concourse.bass_utils import run_bass_kernel_spmd

    x = np.ascontiguousarray(np.asarray(trf, dtype=np.float32)).reshape(NCORES, BL, 12)
    in_maps = [{"trf": x[i]} for i in range(NCORES)]
    nc = _get_nc()
    res = run_bass_kernel_spmd(
        nc, in_maps, list(range(NCORES)), trace=trace, **spmd_kwargs
    )
    outs = np.stack([np.asarray(res.results[i]["out"]) for i in range(NCORES)])
    return outs.reshape(B, 3, 4).astype(np.float32), res


def kernel(trf):
    return run(trf)[0]



# revision 2
# speedup vs baseline: 1.0297x; 1.0297x over previous
"""Trainium2 Bass kernel v5: batched inverse of homogeneous affine transforms.

Host-side data marshalling does the layout work:
  - diag+1 folded into the input on host (A = I + dA)
  - input transposed to column-major SoA planes per core: [12, BL] with
    plane order [a2(3), a3(3), a1(3), t(3)]  (cols 1,2,0,3 of A|t)
  - output written as [12, BL] planes [inv row-major (9), col3 (3)],
    un-transposed on host.

Device: every binary op is a FLAT contiguous DVE op (GPSIMD degrades DVE
when overlapped -> unused); ACT does the unary copies (y-strips, tneg9,
rdet9) in parallel. DMA moves 2KB-contiguous plane runs per partition.

Math (cross-product adjugate):
  x_r = [a2,a3,a1][r] = tin block r; y_r = x_{r+1} = ytile block r
  P[3r+j] = x_r[(j+1)%3] * y_r[(j+2)%3], Q[3r+j] = x_r[(j+2)%3]*y_r[(j+1)%3]
  Z = P-Q (adjugate rows), det = a2 . Z[3..5], inv = Z*rdet,
  col3 = (sum_j Z[3r+j]*(-t_j)) * rdet
"""

import numpy as np

B = 4_194_304
NCORES = 8
BL = B // NCORES
P = 128
C = 512
TMP_BUFS = 1


def _V(base_ap, off, dims):
    import concourse.bass as bass

    return bass.AP(
        base_ap.tensor,
        base_ap.offset + off,
        [list(base_ap.ap[0])] + [[int(s), int(n)] for s, n in dims],
    )


def build_nc(bl=BL, c=C, tmp_bufs=TMP_BUFS):
    import concourse.bass as bass
    import concourse.bacc as bacc
    import concourse.mybir as mybir
    from concourse.tile import TileContext

    f32 = mybir.dt.float32
    nch = bl // (P * c)
    assert bl == nch * P * c

    nc = bacc.Bacc()
    trf = nc.declare_dram_parameter("trf", [12, bl], f32, isOutput=False)
    out = nc.declare_dram_parameter("out", [12, bl], f32, isOutput=True)

    def dram_ap(t, n):
        # chunk n: partition p gets, for each plane e, elems
        # [n*P*c + p*c, +c) of that plane -> free dims (e, k), 2KB runs
        return bass.AP(t.ap().tensor, n * P * c,
                       [[c, P], [bl, 12], [1, c]])

    V = nc.vector
    with TileContext(nc) as tc:
        with (
            tc.tile_pool(name="io", bufs=2) as io,
            tc.tile_pool(name="tmp", bufs=tmp_bufs) as tmp,
            tc.tile_pool(name="pqp", bufs=2) as pqp,
        ):
            for n in range(nch):
                tin = io.tile([P, 12 * c], f32, tag="tin")
                nc.sync.dma_start(out=tin[:], in_=dram_ap(trf, n))

                # y-strips: [a3, a1, a2] = tin[3C:9C] ++ tin[0:3C]
                ytile = tmp.tile([P, 9 * c], f32, tag="ytile")
                nc.scalar.copy(_V(ytile, 0, [(1, 6 * c)]),
                               _V(tin, 3 * c, [(1, 6 * c)]))
                nc.scalar.copy(_V(ytile, 6 * c, [(1, 3 * c)]),
                               _V(tin, 0, [(1, 3 * c)]))

                # tneg9 = [-t, -t, -t]
                tneg9 = tmp.tile([P, 9 * c], f32, tag="tneg9")
                for r in range(3):
                    nc.scalar.mul(_V(tneg9, 3 * r * c, [(1, 3 * c)]),
                                  _V(tin, 9 * c, [(1, 3 * c)]), -1.0)

                # P, Q: one flat-strided op per j, iterating (r, k)
                # Q and u use tout[0:9C] as scratch (overwritten by scale)
                pq = pqp.tile([P, 9 * c], f32, tag="pq")
                tout = io.tile([P, 12 * c], f32, tag="tout")
                qq = tout
                for j in range(3):
                    V.tensor_mul(
                        _V(pq, j * c, [(3 * c, 3), (1, c)]),
                        _V(tin, ((j + 1) % 3) * c, [(3 * c, 3), (1, c)]),
                        _V(ytile, ((j + 2) % 3) * c, [(3 * c, 3), (1, c)]),
                    )
                for j in range(3):
                    V.tensor_mul(
                        _V(qq, j * c, [(3 * c, 3), (1, c)]),
                        _V(tin, ((j + 2) % 3) * c, [(3 * c, 3), (1, c)]),
                        _V(ytile, ((j + 1) % 3) * c, [(3 * c, 3), (1, c)]),
                    )

                # Z = P - Q (flat, in place)
                V.tensor_sub(_V(pq, 0, [(1, 9 * c)]),
                             _V(pq, 0, [(1, 9 * c)]),
                             _V(qq, 0, [(1, 9 * c)]))

                # det = a2 . Z[3..5]  (a2 = tin block 0; all flat)
                tm = tmp.tile([P, 3 * c], f32, tag="tm")
                V.tensor_mul(_V(tm, 0, [(1, 3 * c)]),
                             _V(tin, 0, [(1, 3 * c)]),
                             _V(pq, 3 * c, [(1, 3 * c)]))
                d1 = tmp.tile([P, c], f32, tag="d1")
                det = tmp.tile([P, c], f32, tag="det")
                V.tensor_add(d1[:], _V(tm, 0, [(1, c)]), _V(tm, c, [(1, c)]))
                V.tensor_add(det[:], d1[:], _V(tm, 2 * c, [(1, c)]))

                # rdet -> rdet9 plane 0; ACT replicates to 9 planes
                rdet9 = tmp.tile([P, 9 * c], f32, tag="rdet9")
                V.reciprocal_approx_fast(_V(rdet9, 0, [(1, c)]), det[:])
                nc.scalar.copy(_V(rdet9, c, [(1, c)]), _V(rdet9, 0, [(1, c)]))
                nc.scalar.copy(_V(rdet9, 2 * c, [(1, 2 * c)]),
                               _V(rdet9, 0, [(1, 2 * c)]))
                nc.scalar.copy(_V(rdet9, 4 * c, [(1, 4 * c)]),
                               _V(rdet9, 0, [(1, 4 * c)]))
                nc.scalar.copy(_V(rdet9, 8 * c, [(1, c)]), _V(rdet9, 0, [(1, c)]))

                # u = Z * tneg9 (flat 9C into qq); col3 sums; scale
                V.tensor_mul(_V(qq, 0, [(1, 9 * c)]),
                             _V(pq, 0, [(1, 9 * c)]),
                             _V(tneg9, 0, [(1, 9 * c)]))
                e1 = tmp.tile([P, 3 * c], f32, tag="e1")
                V.tensor_add(_V(e1, 0, [(c, 3), (1, c)]),
                             _V(qq, 0, [(3 * c, 3), (1, c)]),
                             _V(qq, c, [(3 * c, 3), (1, c)]))
                V.tensor_add(_V(e1, 0, [(c, 3), (1, c)]),
                             _V(e1, 0, [(c, 3), (1, c)]),
                             _V(qq, 2 * c, [(3 * c, 3), (1, c)]))

                V.tensor_mul(_V(tout, 0, [(1, 9 * c)]),
                             _V(pq, 0, [(1, 9 * c)]),
                             _V(rdet9, 0, [(1, 9 * c)]))
                V.tensor_mul(_V(tout, 9 * c, [(1, 3 * c)]),
                             _V(e1, 0, [(1, 3 * c)]),
                             _V(rdet9, 0, [(1, 3 * c)]))

                nc.sync.dma_start(out=dram_ap(out, n), in_=tout[:])

    return nc


_CACHE = {}


def _get_nc():
    if "nc" not in _CACHE:
        nc = build_nc()
        nc.finalize()
        _CACHE["nc"] = nc
    return _CACHE["nc"]


def _prep_inputs(trf):
    x = np.asarray(trf, dtype=np.float32).reshape(B, 3, 4).copy()
    x[:, 0, 0] += 1.0
    x[:, 1, 1] += 1.0
    x[:, 2, 2] += 1.0
    # (core, k, row, col) -> (core, col, row, k), cols reordered [1,2,0,3]
    xt = x.reshape(NCORES, BL, 3, 4).transpose(0, 3, 2, 1)[:, [1, 2, 0, 3]]
    return np.ascontiguousarray(xt.reshape(NCORES, 12, BL))


def _decode_outputs(outs):
    # outs: (NCORES, 12, BL): planes inv[3r+j] (row-major) + col3[r]
    inv = outs[:, :9].reshape(NCORES, 3, 3, BL)
    col3 = outs[:, 9:12]
    res = np.empty((NCORES, BL, 3, 4), np.float32)
    res[..., :3] = inv.transpose(0, 3, 1, 2)
    res[..., 3] = col3.transpose(0, 2, 1)
    return res.reshape(B, 3, 4)


def run(trf, trace=False, **spmd_kwargs):
    from concourse.bass_utils import run_bass_kernel_spmd

    xin = _prep_inputs(trf)
    in_maps = [{"trf": xin[i]} for i in range(NCORES)]
    nc = _get_nc()
    res = run_bass_kernel_spmd(
        nc, in_maps, list(range(NCORES)), trace=trace, **spmd_kwargs
    )
    outs = np.stack([np.asarray(res.results[i]["out"]) for i in range(NCORES)])
    return _decode_outputs(outs), res


def kernel(trf):
    return run(trf)[0]
